# revision 11
# baseline (speedup 1.0000x reference)
"""DeepseekV3-style SwiGLU MLP with block-dequantized weights on 8 Trainium2
NeuronCores.

Math (per reference):
    wg = gate_weight * blockscale(gate_scale)   # [I, H], 128x128 blocks
    wu = up_weight   * blockscale(up_scale)
    wd = down_weight * blockscale(down_scale)
    gate = x @ wg.T        # [T, I]
    up   = x @ wu.T
    h    = silu(gate) * up
    out  = h @ wd          # [T, H]

Sharding: tensor-parallel over the intermediate dim I across 8 cores
(column-parallel gate/up, row-parallel down). Each core writes its full
[T, H] f32 partial of the down projection; the host sums the 8 partials
(the "all-reduce" of the RowParallelLinear, done at gather time).

Weights are block-dequantized ON THE HOST (scale folded in) and shipped as
bf16 (halves HBM traffic vs f32; rel err ~4e-3 end to end).  With bf16 the
kernel is TensorE-bound: 1056 N=512 matmuls/core = ~228 us of PE streaming
at 2.4 GHz, vs ~130 us of DMA.

v2 restructure (vs the 315 us v1): phase 1 processes i-tiles in PAIRS,
iterating h-halves outermost:  [g(a) u(a) g(b) u(b)] x hf(2).  A pair's
64+64 matmuls consume xt (4 MB) + 4 MB of weights over 27.6 us = 290 GB/s,
which fits under the ~358 GB/s per-core HBM ceiling -- v1's per-ib order
needed 434 GB/s for its first pass and stalled ~9 us at the head and 5 us
at ib1 (TimelineSim).  The first pair's hf0 weight tiles are split
[P,4,P]+[P,12,P] so the first real matmul is data-ready right as the 8-MM
PE warm burst ends (~3.4 us).  Weight DMAs for pair p+1 issue at pair p's
start (1-pair lookahead, 2 pairs resident).  Phase 2 keeps (q, t) units of
2 PSUM banks pipelined 4 deep, but evacuates both banks into ONE [P,1024]
f32 SBUF tile and writes out with ONE 512 KB DMA (16 big DMAs instead of
32); down-weights for q0/q1 prefetch during phase 1 on the weight rings,
q2/q3 fetch on the scalar ring while out-writes own the sync ring.

Layouts (host prepares in numpy, bf16 = ml_dtypes.bfloat16):
  xt  [P, HB, T]            xt[p, hb, t]           = x[t, hb*128+p]
  wgu [IB, 2, 2, P, 16, P]  wgu[ib, gu, hf, p,o,i] = w_{g/u}[ib*128+i, (hf*16+o)*128+p]
  wd  [NQ, P, IB, HQ]       wd[q, p, ib, j]        = w_d[ib*128+p, q*1024+j]
All are per-partition contiguous for their DMA slices.
"""

import os

import numpy as np

P = 128
T = 512
H = 4096
I_FULL = 11008
NCORES = 8
IB = 11                 # 128-row i-blocks per core (padded 86 -> 88 blocks)
I_CORE = IB * P         # 1408
I_PAD = NCORES * I_CORE  # 11264
HB = H // P             # 32
HCW = 16                # hb per weight tile
NQ = 4                  # down-proj output column quarters
HQ = H // NQ            # 1024
TT = T // P             # 4
WD_GRP = [(0, 4), (4, 4), (8, 3)]  # phase-2 i-tile DMA groups
# Phase-1 i-tile groups: first a TRIPLE so the head consumes bytes at
# 242 GB/s (under the ~358 GB/s HBM ceiling) while xt streams in, then
# doubles.  Each group's 4 psum banks (+6 for the triple) fit the 8-bank
# PSUM with the previous group's banks still evacuating.
GROUPS = [(0, 1, 2), (3, 4), (5, 6), (7, 8), (9, 10)]

LAST_RESULTS = None  # BassKernelResults from the most recent run (for test.py)
_PROG_CACHE = {}     # loop_n -> lowered Bass program


def _build_program(loop_n: int = 1, wgu_bufs: int = 16, wd_bufs: int = 6,
                   warm: int = 8):
    import contextlib

    import concourse.mybir as mybir
    from concourse import bacc
    from concourse.bass import ds, ts
    from concourse.tile import TileContext

    f32 = mybir.dt.float32
    bf16 = mybir.dt.bfloat16
    AF = mybir.ActivationFunctionType
    ALU = mybir.AluOpType

    nc = bacc.Bacc("TRN2", num_devices=NCORES)

    xt = nc.dram_tensor("xt", [P, HB, T], bf16, kind="ExternalInput")
    wgu = nc.dram_tensor("wgu", [IB, 2, 2, P, HCW, P], bf16,
                         kind="ExternalInput")
    wd = nc.dram_tensor("wd", [NQ, P, IB, HQ], bf16, kind="ExternalInput")
    out = nc.dram_tensor("out", [T, H], f32, kind="ExternalOutput")

    with TileContext(nc) as tc:
        with (
            tc.tile_pool(name="const", bufs=1) as cpool,
            tc.tile_pool(name="wgup", bufs=wgu_bufs) as wgu_pool,
            tc.tile_pool(name="wdp", bufs=wd_bufs) as wd_pool,
            tc.tile_pool(name="silp", bufs=2) as sil_pool,
            tc.tile_pool(name="oevp", bufs=4) as oev_pool,
            tc.tile_pool(name="psum", bufs=8, space="PSUM") as ps_pool,
        ):
            loop_cm = (
                tc.For_i(0, loop_n, 1) if loop_n > 1 else contextlib.nullcontext()
            )
            loop_cm.__enter__()

            # PE pre-warm: the HAM clock gate holds TensorE at 1.2 GHz until
            # it has seen ~3.4 us of sustained activity.  8 cold matmuls on a
            # memset tile span that window while the head DMAs stream, so the
            # first real matmul enters at 2.4 GHz.
            xt_sb = cpool.tile([P, HB, T], bf16)
            h_all = cpool.tile([P, IB, T], bf16)

            if warm:
                # Warm matmuls on UNINITIALIZED SBUF (h_all, written much
                # later by phase 1) — garbage x garbage into a discarded
                # psum.  Skipping the memset lets the warm burst start
                # ~1.4 us earlier, right at program start.
                ps_w = ps_pool.tile([P, T], f32, tag="ps")
                for i in range(warm):
                    nc.tensor.matmul(ps_w[:], h_all[:, 0, ds(0, P)],
                                     h_all[:, 0, :],
                                     start=(i == 0), stop=(i == warm - 1))
                wsink = sil_pool.tile([P, T], f32, tag="warm_sink")
                nc.scalar.copy(wsink[:], ps_w[:])

            # ---- weight-tile bookkeeping -------------------------------
            # wtiles[(gu, ib, hf)] -> list of (tile, o_start, o_len)
            wtiles = {}

            def emit_wtile(eng, gu, ib, hf, o0=0, olen=HCW, tag="wgu",
                           bufs=None):
                name = f"w{'gu'[gu]}{ib}_{hf}_{o0}"
                t = wgu_pool.tile([P, olen, P], bf16, tag=tag, name=name,
                                  bufs=bufs)
                eng.dma_start(t[:], wgu[ib, gu, hf, :, ds(o0, olen), :])
                wtiles.setdefault((gu, ib, hf), []).append((t, o0, olen))

            def wslice(gu, ib, hf, o):
                for t, s, ln in wtiles[(gu, ib, hf)]:
                    if s <= o < s + ln:
                        return t[:, o - s]
                raise KeyError((gu, ib, hf, o))

            def xt_chunk(eng, xc):
                eng.dma_start(xt_sb[:, ds(xc * 4, 4), :], xt[:, ds(xc * 4, 4), :])

            S, C = nc.sync, nc.scalar

            # Head DMA schedule, ordered by first consumption (alternating
            # rings).  Pair 0's hf0 tiles are split [0:4)+[4:16) so the first
            # gate matmul is ready at ~3.4 us; xt chunks 0-3 (hb0-15) are
            # needed through the whole hf0 half, chunks 4-7 during hf1.
            # Head: gate(ib0)'s pieces + xt first (the first block's burst),
            # then the rest of the triple's hf0 tiles, then hf1 + late xt.
            emit_wtile(S, 0, 0, 0, 0, 4, tag="wgu0a", bufs=4)
            xt_chunk(C, 0)
            emit_wtile(S, 0, 0, 0, 4, HCW - 4, tag="wgu0b", bufs=4)
            xt_chunk(C, 1)
            xt_chunk(S, 2)
            xt_chunk(C, 3)
            emit_wtile(S, 1, 0, 0, 0, 4, tag="wgu0a", bufs=4)
            emit_wtile(C, 1, 0, 0, 4, HCW - 4, tag="wgu0b", bufs=4)
            emit_wtile(S, 0, 1, 0, 0, 4, tag="wgu0a", bufs=4)
            emit_wtile(C, 0, 1, 0, 4, HCW - 4, tag="wgu0b", bufs=4)
            emit_wtile(S, 1, 1, 0, 0, 4, tag="wgu0a", bufs=4)
            emit_wtile(C, 1, 1, 0, 4, HCW - 4, tag="wgu0b", bufs=4)
            emit_wtile(S, 0, 2, 0)
            emit_wtile(C, 1, 2, 0)
            xt_chunk(S, 4)
            emit_wtile(C, 0, 0, 1)
            xt_chunk(S, 5)
            emit_wtile(C, 1, 0, 1)
            xt_chunk(S, 6)
            emit_wtile(C, 0, 1, 1)
            xt_chunk(S, 7)
            emit_wtile(C, 1, 1, 1)
            emit_wtile(S, 0, 2, 1)
            emit_wtile(C, 1, 2, 1)

            def emit_group_dmas(group):
                for hf in range(2):
                    for ib in group:
                        emit_wtile(S, 0, ib, hf)
                        emit_wtile(C, 1, ib, hf)

            emit_group_dmas(GROUPS[1])

            # Phase-2 wd tiles: wd_tiles[q] = [(g0, sz, tile), ...]
            wd_tiles = {}

            def emit_wd_dmas(eng, q):
                lst = []
                for g0, sz in WD_GRP:
                    dt_ = wd_pool.tile([P, 4, HQ], bf16, tag="wd",
                                       name=f"wd{q}_{g0}")[:, :sz, :]
                    eng.dma_start(dt_, wd[q, :, ds(g0, sz), :])
                    lst.append((g0, sz, dt_))
                wd_tiles[q] = lst

            # ---- phase 1: gate/up projections + SwiGLU -----------------
            for p, group in enumerate(GROUPS):
                if 1 <= p < len(GROUPS) - 1:
                    emit_group_dmas(GROUPS[p + 1])
                if p == 2:
                    emit_wd_dmas(S, 0)
                if p == 3:
                    emit_wd_dmas(C, 1)
                ps = {ib: (ps_pool.tile([P, T], f32, tag="ps",
                                        name=f"ps_g{ib}"),
                           ps_pool.tile([P, T], f32, tag="ps",
                                        name=f"ps_u{ib}"))
                      for ib in group}
                for hf in range(2):
                    for ib in group:
                        for gu in range(2):
                            for o in range(HCW):
                                hb = hf * HCW + o
                                nc.tensor.matmul(
                                    ps[ib][gu][:], wslice(gu, ib, hf, o),
                                    xt_sb[:, hb],
                                    start=(hb == 0), stop=(hb == HB - 1),
                                )
                for ib in group:
                    sil = sil_pool.tile([P, T], f32, tag="sil")
                    nc.scalar.activation(sil[:], ps[ib][0][:], AF.Silu)
                    nc.vector.tensor_tensor(h_all[:, ib, :], sil[:],
                                            ps[ib][1][:], ALU.mult)

            # ---- phase 2: down projection (partial sums to DRAM) -------
            # (q, t) units of 2 PSUM banks, 4 units pipelined via the 8-slot
            # psum ring.  Out-writes own the sync ring; q2/q3 wd fetches ride
            # the scalar ring one q ahead (their slots free exactly then).
            for q in range(NQ):
                if q == 1:
                    emit_wd_dmas(C, 2)
                if q == 2:
                    emit_wd_dmas(C, 3)
                wdq = wd_tiles[q]
                for t in range(TT):
                    ps_o = [
                        ps_pool.tile([P, 512], f32, tag="ps",
                                     name=f"ps_o_{q}_{t}_{hc}")
                        for hc in range(2)
                    ]
                    for hc in range(2):
                        for g0, sz, dt_ in wdq:
                            for k in range(sz):
                                ib = g0 + k
                                nc.tensor.matmul(
                                    ps_o[hc][:],
                                    h_all[:, ib, ts(t, P)],
                                    dt_[:, k, ds(hc * 512, 512)],
                                    start=(ib == 0),
                                    stop=(ib == IB - 1),
                                )
                    if q == NQ - 1 and t == TT - 1:
                        # Last unit: hc0 (done 11 MMs early) evacuates in
                        # 256-quarters under the hc1 MMs; hc1 in one half so
                        # the exposed tail is one ACT copy + one 256 KB DMA.
                        for hc4 in range(2):
                            otq = oev_pool.tile([P, 256], f32, tag="oevq",
                                                bufs=4, name=f"otq_{hc4}")
                            nc.scalar.copy(
                                otq[:], ps_o[0][:, ds(hc4 * 256, 256)])
                            eng = S if hc4 % 2 == 0 else C
                            eng.dma_start(
                                out[ds(t * P, P),
                                    ds(q * HQ + hc4 * 256, 256)], otq[:])
                        oth = oev_pool.tile([P, 512], f32, tag="oevq",
                                            bufs=4, name="oth_last")
                        nc.scalar.copy(oth[:], ps_o[1][:])
                        nc.sync.dma_start(
                            out[ds(t * P, P), ds(q * HQ + 512, 512)], oth[:])
                    else:
                        ot = oev_pool.tile([P, 2 * 512], f32, tag="oev",
                                           name=f"ot_{q}_{t}")
                        for hc in range(2):
                            # ACT copy: DVE tensor_copy measured faster in
                            # the cost model but hit NRT_EXEC_UNIT_
                            # UNRECOVERABLE on hardware; ACT is the
                            # verified-stable path.
                            nc.scalar.copy(ot[:, ds(hc * 512, 512)],
                                           ps_o[hc][:])
                        nc.sync.dma_start(
                            out[ds(t * P, P), ds(q * HQ, HQ)], ot[:]
                        )

            loop_cm.__exit__(None, None, None)

    nc.compile()  # bacc lowering: register alloc + multi-wait splitting
    return nc


def _prep_inputs(x, gate_weight, up_weight, down_weight, gate_scale, up_scale,
                 down_scale):
    """Dequantize + pad + shard + transpose on the host into per-core bf16
    DMA layouts (see module docstring)."""
    import ml_dtypes

    bf = ml_dtypes.bfloat16

    def deq_pad(w, s):
        w = np.asarray(w, np.float32)
        s = np.asarray(s, np.float32)
        wd_ = (w.reshape(I_FULL // P, P, HB, P) * s[:, None, :, None]).reshape(
            I_FULL, H
        ).astype(bf)
        wp = np.zeros((I_PAD, H), bf)
        wp[:I_FULL] = wd_
        return wp

    gw = deq_pad(gate_weight, gate_scale)
    uw = deq_pad(up_weight, up_scale)
    dw = deq_pad(down_weight, down_scale)

    x = np.asarray(x, np.float32).astype(bf)
    # xt[p, hb, t] = x[t, hb*128+p]
    xt = np.ascontiguousarray(x.reshape(T, HB, P).transpose(2, 1, 0))

    in_maps = []
    for c in range(NCORES):
        i0 = c * I_CORE

        # [ib, i, hb', p] -> [ib, hb, p, i] -> [ib, hf, p, o, i]
        def gu_prep(wc):
            a = wc.reshape(IB, P, HB, P).transpose(0, 2, 3, 1)
            a = a.reshape(IB, 2, HCW, P, P).transpose(0, 1, 3, 2, 4)
            return a

        g5 = gu_prep(gw[i0: i0 + I_CORE])
        u5 = gu_prep(uw[i0: i0 + I_CORE])
        wgu_prep = np.ascontiguousarray(
            np.stack([g5, u5], axis=1)  # [ib, gu, hf, p, o, i]
        )
        # down: [q, p, ib, j] = w[ib*128+p, q*1024+j]
        wd_prep = np.ascontiguousarray(
            dw[i0: i0 + I_CORE].reshape(IB, P, NQ, HQ).transpose(2, 1, 0, 3)
        )
        in_maps.append({"xt": xt, "wgu": wgu_prep, "wd": wd_prep})
    return in_maps


def kernel(x, gate_weight, up_weight, down_weight, gate_scale, up_scale,
           down_scale, blocksize):
    global LAST_RESULTS
    assert int(blocksize) == P, f"kernel hardcodes blocksize=128, got {blocksize}"

    from concourse.bass_utils import run_bass_kernel_spmd

    trace = os.environ.get("BASS_TRACE", "0") == "1"

    nc = _PROG_CACHE.get(1)
    if nc is None:
        nc = _build_program()
        _PROG_CACHE[1] = nc
    in_maps = _prep_inputs(
        x, gate_weight, up_weight, down_weight, gate_scale, up_scale, down_scale
    )
    results = run_bass_kernel_spmd(
        nc, in_maps, core_ids=list(range(NCORES)), trace=trace
    )
    LAST_RESULTS = results

    acc = np.zeros((T, H), np.float64)
    for res in results.results:
        acc += res["out"]
    return acc.astype(np.float32)


# revision 25
# speedup vs baseline: 1.0862x; 1.0862x over previous
"""DeepseekV3-style SwiGLU MLP with block-dequantized weights on 8 Trainium2
NeuronCores.

Math (per reference):
    wg = gate_weight * blockscale(gate_scale)   # [I, H], 128x128 blocks
    wu = up_weight   * blockscale(up_scale)
    wd = down_weight * blockscale(down_scale)
    gate = x @ wg.T        # [T, I]
    up   = x @ wu.T
    h    = silu(gate) * up
    out  = h @ wd          # [T, H]

Sharding: tensor-parallel over the intermediate dim I across 8 cores
(column-parallel gate/up, row-parallel down). Each core writes its full
[T, H] f32 partial of the down projection; the host sums the 8 partials
(the "all-reduce" of the RowParallelLinear, done at gather time).

Weights are block-dequantized ON THE HOST (scale folded in) and shipped as
bf16 (halves HBM traffic vs f32; rel err ~4e-3 end to end).  With bf16 the
kernel is TensorE-bound: 1056 N=512 matmuls/core = ~228 us of PE streaming
at 2.4 GHz, vs ~130 us of DMA.

v2 restructure (vs the 315 us v1): phase 1 processes i-tiles in GROUPS
(a triple, then doubles), iterating h-halves outermost:
[g(a) u(a) g(b) u(b) ...] x hf(2).  The triple's 192 matmuls consume
xt (4 MB) + 6 MB of weights over 41.4 us = 242 GB/s, under the ~358 GB/s
per-core HBM ceiling -- v1's per-ib order needed 434 GB/s for its first
pass and stalled ~9 us at the head plus 5 us at ib1 (TimelineSim), a
>3.4us PE gap that also re-throttled the HAM clock gate mid-kernel.  The
first tiles are split [P,4,P]+[P,12,P] so the first real matmul is
data-ready right as the 8-MM PE warm burst ends (~3.4 us).  Weight DMAs
for group p+1 issue at group p's start (2 groups resident).  Phase 2
keeps (q, t) units of 2 PSUM banks pipelined 4 deep, but evacuates both
banks into ONE [P,1024] SBUF tile and writes out with ONE 512 KB DMA
(16+2 DMAs instead of 32); down-weights for q0/q1 prefetch during
phase 1 on the weight rings, q2/q3 fetch on the scalar ring while
out-writes own the sync ring; the last unit evacuates in small pieces so
the exposed tail is ~2 us.  TimelineSim single pass: 239.3 us (PE busy
228.8 us = the bf16 2.4 GHz streaming floor for 1056 N=512 matmuls + 8
warm; PE idle only ~6 us of head DMA-phasing + ~4 us tail).  Measured
loop-slope (For_i body, quiet box): ~310-335 us/iter vs v1's ~375-440
in the same sessions; on a power-throttled box both scale ~2x (PE held
at 1.2 GHz) with the v2 advantage intact (-32 us median).

Layouts (host prepares in numpy, bf16 = ml_dtypes.bfloat16):
  xt  [P, HB, T]            xt[p, hb, t]           = x[t, hb*128+p]
  wgu [IB, 2, 2, P, 16, P]  wgu[ib, gu, hf, p,o,i] = w_{g/u}[ib*128+i, (hf*16+o)*128+p]
  wd  [NQ, P, IB, HQ]       wd[q, p, ib, j]        = w_d[ib*128+p, q*1024+j]
All are per-partition contiguous for their DMA slices.
"""

import os

import numpy as np

P = 128
T = 512
H = 4096
I_FULL = 11008
NCORES = 8
IB = 11                 # 128-row i-blocks per core (padded 86 -> 88 blocks)
I_CORE = IB * P         # 1408
I_PAD = NCORES * I_CORE  # 11264
HB = H // P             # 32
HCW = 16                # hb per weight tile
NQ = 4                  # down-proj output column quarters
HQ = H // NQ            # 1024
TT = T // P             # 4
WD_GRP = [(0, 4), (4, 4), (8, 3)]  # phase-2 i-tile DMA groups
# Phase-1 i-tile groups: first a TRIPLE so the head consumes bytes at
# 242 GB/s (under the ~358 GB/s HBM ceiling) while xt streams in, then
# doubles.  Each group's 4 psum banks (+6 for the triple) fit the 8-bank
# PSUM with the previous group's banks still evacuating.
GROUPS = [(0, 1, 2), (3, 4), (5, 6), (7, 8), (9, 10)]

LAST_RESULTS = None  # BassKernelResults from the most recent run (for test.py)
_PROG_CACHE = {}     # loop_n -> lowered Bass program


def _build_program(loop_n: int = 1, wgu_bufs: int = 16, wd_bufs: int = 6,
                   warm: int = 8, phases: str = "12", out_bf16: int = 1):
    import contextlib

    import concourse.mybir as mybir
    from concourse import bacc
    from concourse.bass import ds, ts
    from concourse.tile import TileContext

    f32 = mybir.dt.float32
    bf16 = mybir.dt.bfloat16
    AF = mybir.ActivationFunctionType
    ALU = mybir.AluOpType

    nc = bacc.Bacc("TRN2", num_devices=NCORES)

    odt = bf16 if out_bf16 else f32
    xt = nc.dram_tensor("xt", [P, HB, T], bf16, kind="ExternalInput")
    wgu = nc.dram_tensor("wgu", [IB, 2, 2, P, HCW, P], bf16,
                         kind="ExternalInput")
    wd = nc.dram_tensor("wd", [NQ, P, IB, HQ], bf16, kind="ExternalInput")
    out = nc.dram_tensor("out", [T, H], odt, kind="ExternalOutput")

    with TileContext(nc) as tc:
        with (
            tc.tile_pool(name="const", bufs=1) as cpool,
            tc.tile_pool(name="wgup", bufs=wgu_bufs) as wgu_pool,
            tc.tile_pool(name="wdp", bufs=wd_bufs) as wd_pool,
            tc.tile_pool(name="silp", bufs=2) as sil_pool,
            tc.tile_pool(name="oevp", bufs=4) as oev_pool,
            tc.tile_pool(name="psum", bufs=8, space="PSUM") as ps_pool,
        ):
            loop_cm = (
                tc.For_i(0, loop_n, 1) if loop_n > 1 else contextlib.nullcontext()
            )
            loop_cm.__enter__()

            # PE pre-warm: the HAM clock gate holds TensorE at 1.2 GHz until
            # it has seen ~3.4 us of sustained activity.  8 cold matmuls on a
            # memset tile span that window while the head DMAs stream, so the
            # first real matmul enters at 2.4 GHz.
            xt_sb = cpool.tile([P, HB, T], bf16)
            h_all = cpool.tile([P, IB, T], bf16)

            if warm:
                # Warm matmuls on UNINITIALIZED SBUF (h_all, written much
                # later by phase 1) — garbage x garbage into a discarded
                # psum.  Skipping the memset lets the warm burst start
                # ~1.4 us earlier, right at program start.
                ps_w = ps_pool.tile([P, T], f32, tag="ps")
                for i in range(warm):
                    nc.tensor.matmul(ps_w[:], h_all[:, 0, ds(0, P)],
                                     h_all[:, 0, :],
                                     start=(i == 0), stop=(i == warm - 1))
                wsink = sil_pool.tile([P, T], f32, tag="warm_sink")
                nc.scalar.copy(wsink[:], ps_w[:])

            # ---- weight-tile bookkeeping -------------------------------
            # wtiles[(gu, ib, hf)] -> list of (tile, o_start, o_len)
            wtiles = {}

            def emit_wtile(eng, gu, ib, hf, o0=0, olen=HCW, tag="wgu",
                           bufs=None):
                name = f"w{'gu'[gu]}{ib}_{hf}_{o0}"
                t = wgu_pool.tile([P, olen, P], bf16, tag=tag, name=name,
                                  bufs=bufs)
                eng.dma_start(t[:], wgu[ib, gu, hf, :, ds(o0, olen), :])
                wtiles.setdefault((gu, ib, hf), []).append((t, o0, olen))

            def wslice(gu, ib, hf, o):
                for t, s, ln in wtiles[(gu, ib, hf)]:
                    if s <= o < s + ln:
                        return t[:, o - s]
                raise KeyError((gu, ib, hf, o))

            def xt_chunk(eng, xc):
                eng.dma_start(xt_sb[:, ds(xc * 4, 4), :], xt[:, ds(xc * 4, 4), :])

            S, C = nc.sync, nc.scalar

            # Head DMA schedule, ordered by first consumption (alternating
            # rings).  Pair 0's hf0 tiles are split [0:4)+[4:16) so the first
            # gate matmul is ready at ~3.4 us; xt chunks 0-3 (hb0-15) are
            # needed through the whole hf0 half, chunks 4-7 during hf1.
            # Head: gate(ib0)'s pieces + xt first (the first block's burst),
            # then the rest of the triple's hf0 tiles, then hf1 + late xt.
            do1, do2 = "1" in str(phases), "2" in str(phases)
            if do1:
                emit_wtile(S, 0, 0, 0, 0, 4, tag="wgu0a", bufs=4)
                xt_chunk(C, 0)
                emit_wtile(S, 0, 0, 0, 4, HCW - 4, tag="wgu0b", bufs=4)
                xt_chunk(C, 1)
                xt_chunk(S, 2)
                xt_chunk(C, 3)
                emit_wtile(S, 1, 0, 0, 0, 4, tag="wgu0a", bufs=4)
                emit_wtile(C, 1, 0, 0, 4, HCW - 4, tag="wgu0b", bufs=4)
                emit_wtile(S, 0, 1, 0, 0, 4, tag="wgu0a", bufs=4)
                emit_wtile(C, 0, 1, 0, 4, HCW - 4, tag="wgu0b", bufs=4)
                emit_wtile(S, 1, 1, 0, 0, 4, tag="wgu0a", bufs=4)
                emit_wtile(C, 1, 1, 0, 4, HCW - 4, tag="wgu0b", bufs=4)
                emit_wtile(S, 0, 2, 0)
                emit_wtile(C, 1, 2, 0)
                xt_chunk(S, 4)
                emit_wtile(C, 0, 0, 1)
                xt_chunk(S, 5)
                emit_wtile(C, 1, 0, 1)
                xt_chunk(S, 6)
                emit_wtile(C, 0, 1, 1)
                xt_chunk(S, 7)
                emit_wtile(C, 1, 1, 1)
                emit_wtile(S, 0, 2, 1)
                emit_wtile(C, 1, 2, 1)

            def emit_group_dmas(group):
                for hf in range(2):
                    for ib in group:
                        emit_wtile(S, 0, ib, hf)
                        emit_wtile(C, 1, ib, hf)

            if do1:
                emit_group_dmas(GROUPS[1])

            # Phase-2 wd tiles: wd_tiles[q] = [(g0, sz, tile), ...]
            wd_tiles = {}

            def emit_wd_dmas(eng, q):
                lst = []
                for g0, sz in WD_GRP:
                    dt_ = wd_pool.tile([P, 4, HQ], bf16, tag="wd",
                                       name=f"wd{q}_{g0}")[:, :sz, :]
                    eng.dma_start(dt_, wd[q, :, ds(g0, sz), :])
                    lst.append((g0, sz, dt_))
                wd_tiles[q] = lst

            if do2 and not do1:
                # phase-2-only (bench diagnostic): h_all needs a writer
                nc.vector.memset(h_all[:], 0.25)
                emit_wd_dmas(S, 0)
                emit_wd_dmas(C, 1)

            # ---- phase 1: gate/up projections + SwiGLU -----------------
            for p, group in enumerate(GROUPS if do1 else ()):
                if 1 <= p < len(GROUPS) - 1:
                    emit_group_dmas(GROUPS[p + 1])
                if p == 2 and do2:
                    emit_wd_dmas(S, 0)
                if p == 3 and do2:
                    emit_wd_dmas(C, 1)
                ps = {ib: (ps_pool.tile([P, T], f32, tag="ps",
                                        name=f"ps_g{ib}"),
                           ps_pool.tile([P, T], f32, tag="ps",
                                        name=f"ps_u{ib}"))
                      for ib in group}
                for hf in range(2):
                    for ib in group:
                        for gu in range(2):
                            for o in range(HCW):
                                hb = hf * HCW + o
                                nc.tensor.matmul(
                                    ps[ib][gu][:], wslice(gu, ib, hf, o),
                                    xt_sb[:, hb],
                                    start=(hb == 0), stop=(hb == HB - 1),
                                )
                for ib in group:
                    sil = sil_pool.tile([P, T], f32, tag="sil")
                    nc.scalar.activation(sil[:], ps[ib][0][:], AF.Silu)
                    nc.vector.tensor_tensor(h_all[:, ib, :], sil[:],
                                            ps[ib][1][:], ALU.mult)

            # ---- phase 2: down projection (partial sums to DRAM) -------
            # (q, t) units of 2 PSUM banks, 4 units pipelined via the 8-slot
            # psum ring.  Out-writes own the sync ring; q2/q3 wd fetches ride
            # the scalar ring one q ahead (their slots free exactly then).
            for q in range(NQ if do2 else 0):
                if q == 1:
                    emit_wd_dmas(C, 2)
                if q == 2:
                    emit_wd_dmas(C, 3)
                wdq = wd_tiles[q]
                for t in range(TT):
                    ps_o = [
                        ps_pool.tile([P, 512], f32, tag="ps",
                                     name=f"ps_o_{q}_{t}_{hc}")
                        for hc in range(2)
                    ]
                    for hc in range(2):
                        for g0, sz, dt_ in wdq:
                            for k in range(sz):
                                ib = g0 + k
                                nc.tensor.matmul(
                                    ps_o[hc][:],
                                    h_all[:, ib, ts(t, P)],
                                    dt_[:, k, ds(hc * 512, 512)],
                                    start=(ib == 0),
                                    stop=(ib == IB - 1),
                                )
                    if q == NQ - 1 and t == TT - 1:
                        # Last unit: hc0 (done 11 MMs early) evacuates in
                        # 256-quarters under the hc1 MMs; hc1 in one half so
                        # the exposed tail is one ACT copy + one small DMA.
                        for hc4 in range(2):
                            otq = oev_pool.tile([P, 256], odt, tag="oevq",
                                                bufs=4, name=f"otq_{hc4}")
                            nc.scalar.copy(
                                otq[:], ps_o[0][:, ds(hc4 * 256, 256)])
                            eng = S if hc4 % 2 == 0 else C
                            eng.dma_start(
                                out[ds(t * P, P),
                                    ds(q * HQ + hc4 * 256, 256)], otq[:])
                        oth = oev_pool.tile([P, 512], odt, tag="oevq",
                                            bufs=4, name="oth_last")
                        nc.scalar.copy(oth[:], ps_o[1][:])
                        nc.sync.dma_start(
                            out[ds(t * P, P), ds(q * HQ + 512, 512)], oth[:])
                    else:
                        ot = oev_pool.tile([P, 2 * 512], odt, tag="oev",
                                           name=f"ot_{q}_{t}")
                        for hc in range(2):
                            # ACT copy: DVE tensor_copy measured faster in
                            # the cost model but hit NRT_EXEC_UNIT_
                            # UNRECOVERABLE on hardware; ACT is the
                            # verified-stable path.
                            nc.scalar.copy(ot[:, ds(hc * 512, 512)],
                                           ps_o[hc][:])
                        nc.sync.dma_start(
                            out[ds(t * P, P), ds(q * HQ, HQ)], ot[:]
                        )

            loop_cm.__exit__(None, None, None)

    nc.compile()  # bacc lowering: register alloc + multi-wait splitting
    return nc


def _prep_inputs(x, gate_weight, up_weight, down_weight, gate_scale, up_scale,
                 down_scale):
    """Dequantize + pad + shard + transpose on the host into per-core bf16
    DMA layouts (see module docstring)."""
    import ml_dtypes

    bf = ml_dtypes.bfloat16

    def deq_pad(w, s):
        w = np.asarray(w, np.float32)
        s = np.asarray(s, np.float32)
        wd_ = (w.reshape(I_FULL // P, P, HB, P) * s[:, None, :, None]).reshape(
            I_FULL, H
        ).astype(bf)
        wp = np.zeros((I_PAD, H), bf)
        wp[:I_FULL] = wd_
        return wp

    gw = deq_pad(gate_weight, gate_scale)
    uw = deq_pad(up_weight, up_scale)
    dw = deq_pad(down_weight, down_scale)

    x = np.asarray(x, np.float32).astype(bf)
    # xt[p, hb, t] = x[t, hb*128+p]
    xt = np.ascontiguousarray(x.reshape(T, HB, P).transpose(2, 1, 0))

    in_maps = []
    for c in range(NCORES):
        i0 = c * I_CORE

        # [ib, i, hb', p] -> [ib, hb, p, i] -> [ib, hf, p, o, i]
        def gu_prep(wc):
            a = wc.reshape(IB, P, HB, P).transpose(0, 2, 3, 1)
            a = a.reshape(IB, 2, HCW, P, P).transpose(0, 1, 3, 2, 4)
            return a

        g5 = gu_prep(gw[i0: i0 + I_CORE])
        u5 = gu_prep(uw[i0: i0 + I_CORE])
        wgu_prep = np.ascontiguousarray(
            np.stack([g5, u5], axis=1)  # [ib, gu, hf, p, o, i]
        )
        # down: [q, p, ib, j] = w[ib*128+p, q*1024+j]
        wd_prep = np.ascontiguousarray(
            dw[i0: i0 + I_CORE].reshape(IB, P, NQ, HQ).transpose(2, 1, 0, 3)
        )
        in_maps.append({"xt": xt, "wgu": wgu_prep, "wd": wd_prep})
    return in_maps


def kernel(x, gate_weight, up_weight, down_weight, gate_scale, up_scale,
           down_scale, blocksize):
    global LAST_RESULTS
    assert int(blocksize) == P, f"kernel hardcodes blocksize=128, got {blocksize}"

    from concourse.bass_utils import run_bass_kernel_spmd

    trace = os.environ.get("BASS_TRACE", "0") == "1"

    nc = _PROG_CACHE.get(1)
    if nc is None:
        nc = _build_program()
        _PROG_CACHE[1] = nc
    in_maps = _prep_inputs(
        x, gate_weight, up_weight, down_weight, gate_scale, up_scale, down_scale
    )
    results = run_bass_kernel_spmd(
        nc, in_maps, core_ids=list(range(NCORES)), trace=trace
    )
    LAST_RESULTS = results

    acc = np.zeros((T, H), np.float64)
    for res in results.results:
        acc += np.asarray(res["out"], np.float64)
    return acc.astype(np.float32)


# revision 29
# speedup vs baseline: 1.1250x; 1.0357x over previous
"""DeepseekV3-style SwiGLU MLP with block-dequantized weights on 8 Trainium2
NeuronCores.

Math (per reference):
    wg = gate_weight * blockscale(gate_scale)   # [I, H], 128x128 blocks
    wu = up_weight   * blockscale(up_scale)
    wd = down_weight * blockscale(down_scale)
    gate = x @ wg.T        # [T, I]
    up   = x @ wu.T
    h    = silu(gate) * up
    out  = h @ wd          # [T, H]

Sharding: tensor-parallel over the intermediate dim I across 8 cores
(column-parallel gate/up, row-parallel down). Each core writes its full
[T, H] bf16 partial of the down projection; the host sums the 8 partials
in f64 (the "all-reduce" of the RowParallelLinear, done at gather time;
bf16 partials cost +3e-4 rel err and halve the output write traffic).

Weights are block-dequantized ON THE HOST (scale folded in) and shipped as
bf16 (halves HBM traffic vs f32; rel err ~4e-3 end to end).  With bf16 the
kernel is TensorE-bound: 1056 N=512 matmuls/core = ~228 us of PE streaming
at 2.4 GHz, vs ~130 us of DMA.

v2 restructure (vs the 315 us v1): phase 1 processes i-tiles in GROUPS
(a triple, then doubles), iterating h-halves outermost:
[g(a) u(a) g(b) u(b) ...] x hf(2).  The triple's 192 matmuls consume
xt (4 MB) + 6 MB of weights over 41.4 us = 242 GB/s, under the ~358 GB/s
per-core HBM ceiling -- v1's per-ib order needed 434 GB/s for its first
pass and stalled ~9 us at the head plus 5 us at ib1 (TimelineSim), a
>3.4us PE gap that also re-throttled the HAM clock gate mid-kernel.  The
first tiles are split [P,4,P]+[P,12,P] so the first real matmul is
data-ready right as the 8-MM PE warm burst ends (~3.4 us).  Weight DMAs
for group p+1 issue at group p's start (2 groups resident).  Phase 2
keeps (q, t) units of 2 PSUM banks pipelined 4 deep, but evacuates both
banks into ONE [P,1024] SBUF tile and writes out with ONE 512 KB DMA
(16+2 DMAs instead of 32); down-weights for q0/q1 prefetch during
phase 1 on the weight rings, q2/q3 fetch on the scalar ring while
out-writes own the sync ring; the last unit evacuates in small pieces so
the exposed tail is ~2 us.  TimelineSim single pass: 239.3 us (PE busy
228.8 us = the bf16 2.4 GHz streaming floor for 1056 N=512 matmuls + 8
warm; PE idle only ~6 us of head DMA-phasing + ~4 us tail).  Measured
loop-slope (For_i body, quiet box): ~310-335 us/iter vs v1's ~375-440
in the same sessions; on a power-throttled box both scale ~2x (PE held
at 1.2 GHz) with the v2 advantage intact (-32 us median).

Layouts (host prepares in numpy, bf16 = ml_dtypes.bfloat16):
  xt  [P, HB, T]            xt[p, hb, t]           = x[t, hb*128+p]
  wgu [IB, 2, 2, P, 16, P]  wgu[ib, gu, hf, p,o,i] = w_{g/u}[ib*128+i, (hf*16+o)*128+p]
  wd  [NQ, P, IB, HQ]       wd[q, p, ib, j]        = w_d[ib*128+p, q*1024+j]
All are per-partition contiguous for their DMA slices.
"""

import os

import numpy as np

P = 128
T = 512
H = 4096
I_FULL = 11008
NCORES = 8
IB = 11                 # 128-row i-blocks per core (padded 86 -> 88 blocks)
I_CORE = IB * P         # 1408
I_PAD = NCORES * I_CORE  # 11264
HB = H // P             # 32
HCW = 16                # hb per weight tile
NQ = 4                  # down-proj output column quarters
HQ = H // NQ            # 1024
TT = T // P             # 4
WD_GRP = [(0, 4), (4, 4), (8, 3)]  # phase-2 i-tile DMA groups
# Phase-1 i-tile groups: first a TRIPLE so the head consumes bytes at
# 242 GB/s (under the ~358 GB/s HBM ceiling) while xt streams in, then
# doubles.  Each group's 4 psum banks (+6 for the triple) fit the 8-bank
# PSUM with the previous group's banks still evacuating.
GROUPS = [(0, 1, 2), (3, 4), (5, 6), (7, 8), (9, 10)]

LAST_RESULTS = None  # BassKernelResults from the most recent run (for test.py)
_PROG_CACHE = {}     # loop_n -> lowered Bass program


def _build_program(loop_n: int = 1, wgu_bufs: int = 16, wd_bufs: int = 6,
                   warm: int = 8, phases: str = "12", out_bf16: int = 1):
    import contextlib

    import concourse.mybir as mybir
    from concourse import bacc
    from concourse.bass import ds, ts
    from concourse.tile import TileContext

    f32 = mybir.dt.float32
    bf16 = mybir.dt.bfloat16
    AF = mybir.ActivationFunctionType
    ALU = mybir.AluOpType

    nc = bacc.Bacc("TRN2", num_devices=NCORES)

    odt = bf16 if out_bf16 else f32
    xt = nc.dram_tensor("xt", [P, HB, T], bf16, kind="ExternalInput")
    wgu = nc.dram_tensor("wgu", [IB, 2, 2, P, HCW, P], bf16,
                         kind="ExternalInput")
    wd = nc.dram_tensor("wd", [NQ, P, IB, HQ], bf16, kind="ExternalInput")
    out = nc.dram_tensor("out", [T, H], odt, kind="ExternalOutput")

    with TileContext(nc) as tc:
        with (
            tc.tile_pool(name="const", bufs=1) as cpool,
            tc.tile_pool(name="wgup", bufs=wgu_bufs) as wgu_pool,
            tc.tile_pool(name="wdp", bufs=wd_bufs) as wd_pool,
            tc.tile_pool(name="silp", bufs=2) as sil_pool,
            tc.tile_pool(name="oevp", bufs=4) as oev_pool,
            tc.tile_pool(name="psum", bufs=8, space="PSUM") as ps_pool,
        ):
            loop_cm = (
                tc.For_i(0, loop_n, 1) if loop_n > 1 else contextlib.nullcontext()
            )
            loop_cm.__enter__()

            # PE pre-warm: the HAM clock gate holds TensorE at 1.2 GHz until
            # it has seen ~3.4 us of sustained activity.  8 cold matmuls
            # span that window while the head DMAs stream, so the first
            # real matmul enters at 2.4 GHz.
            xt_sb = cpool.tile([P, HB, T], bf16)
            h_all = cpool.tile([P, IB, T], bf16)

            if warm:
                # Warm matmuls on UNINITIALIZED SBUF (h_all, written much
                # later by phase 1) — garbage x garbage into a discarded
                # psum.  Skipping the memset lets the warm burst start
                # ~1.4 us earlier, right at program start.
                ps_w = ps_pool.tile([P, T], f32, tag="ps")
                for i in range(warm):
                    nc.tensor.matmul(ps_w[:], h_all[:, 0, ds(0, P)],
                                     h_all[:, 0, :],
                                     start=(i == 0), stop=(i == warm - 1))
                wsink = sil_pool.tile([P, T], f32, tag="warm_sink")
                nc.scalar.copy(wsink[:], ps_w[:])

            # ---- weight-tile bookkeeping -------------------------------
            # wtiles[(gu, ib, hf)] -> list of (tile, o_start, o_len)
            wtiles = {}

            def emit_wtile(eng, gu, ib, hf, o0=0, olen=HCW, tag="wgu",
                           bufs=None):
                name = f"w{'gu'[gu]}{ib}_{hf}_{o0}"
                t = wgu_pool.tile([P, olen, P], bf16, tag=tag, name=name,
                                  bufs=bufs)
                eng.dma_start(t[:], wgu[ib, gu, hf, :, ds(o0, olen), :])
                wtiles.setdefault((gu, ib, hf), []).append((t, o0, olen))

            def wslice(gu, ib, hf, o):
                for t, s, ln in wtiles[(gu, ib, hf)]:
                    if s <= o < s + ln:
                        return t[:, o - s]
                raise KeyError((gu, ib, hf, o))

            def xt_chunk(eng, xc):
                eng.dma_start(xt_sb[:, ds(xc * 4, 4), :], xt[:, ds(xc * 4, 4), :])

            S, C = nc.sync, nc.scalar

            # Head DMA schedule, ordered by first consumption (alternating
            # rings).  Pair 0's hf0 tiles are split [0:4)+[4:16) so the first
            # gate matmul is ready at ~3.4 us; xt chunks 0-3 (hb0-15) are
            # needed through the whole hf0 half, chunks 4-7 during hf1.
            # Head: gate(ib0)'s pieces + xt first (the first block's burst),
            # then the rest of the triple's hf0 tiles, then hf1 + late xt.
            do1, do2 = "1" in str(phases), "2" in str(phases)
            if do1:
                emit_wtile(S, 0, 0, 0, 0, 4, tag="wgu0a", bufs=4)
                xt_chunk(C, 0)
                emit_wtile(S, 0, 0, 0, 4, HCW - 4, tag="wgu0b", bufs=4)
                xt_chunk(C, 1)
                xt_chunk(S, 2)
                xt_chunk(C, 3)
                emit_wtile(S, 1, 0, 0, 0, 4, tag="wgu0a", bufs=4)
                emit_wtile(C, 1, 0, 0, 4, HCW - 4, tag="wgu0b", bufs=4)
                emit_wtile(S, 0, 1, 0, 0, 4, tag="wgu0a", bufs=4)
                emit_wtile(C, 0, 1, 0, 4, HCW - 4, tag="wgu0b", bufs=4)
                emit_wtile(S, 1, 1, 0, 0, 4, tag="wgu0a", bufs=4)
                emit_wtile(C, 1, 1, 0, 4, HCW - 4, tag="wgu0b", bufs=4)
                emit_wtile(S, 0, 2, 0)
                emit_wtile(C, 1, 2, 0)
                xt_chunk(S, 4)
                emit_wtile(C, 0, 0, 1)
                xt_chunk(S, 5)
                emit_wtile(C, 1, 0, 1)
                xt_chunk(S, 6)
                emit_wtile(C, 0, 1, 1)
                xt_chunk(S, 7)
                emit_wtile(C, 1, 1, 1)
                emit_wtile(S, 0, 2, 1)
                emit_wtile(C, 1, 2, 1)

            def emit_group_dmas(group):
                for hf in range(2):
                    for ib in group:
                        emit_wtile(S, 0, ib, hf)
                        emit_wtile(C, 1, ib, hf)

            if do1:
                emit_group_dmas(GROUPS[1])

            # Phase-2 wd tiles: wd_tiles[q] = [(g0, sz, tile), ...]
            wd_tiles = {}

            def emit_wd_dmas(eng, q):
                lst = []
                for g0, sz in WD_GRP:
                    dt_ = wd_pool.tile([P, 4, HQ], bf16, tag="wd",
                                       name=f"wd{q}_{g0}")[:, :sz, :]
                    eng.dma_start(dt_, wd[q, :, ds(g0, sz), :])
                    lst.append((g0, sz, dt_))
                wd_tiles[q] = lst

            if do2 and not do1:
                # phase-2-only (bench diagnostic): h_all needs a writer
                nc.vector.memset(h_all[:], 0.25)
                emit_wd_dmas(S, 0)
                emit_wd_dmas(C, 1)

            # ---- phase 1: gate/up projections + SwiGLU -----------------
            for p, group in enumerate(GROUPS if do1 else ()):
                if 1 <= p < len(GROUPS) - 1:
                    emit_group_dmas(GROUPS[p + 1])
                if p == 2 and do2:
                    emit_wd_dmas(S, 0)
                if p == 3 and do2:
                    emit_wd_dmas(C, 1)
                ps = {ib: (ps_pool.tile([P, T], f32, tag="ps",
                                        name=f"ps_g{ib}"),
                           ps_pool.tile([P, T], f32, tag="ps",
                                        name=f"ps_u{ib}"))
                      for ib in group}
                for hf in range(2):
                    for ib in group:
                        for gu in range(2):
                            for o in range(HCW):
                                hb = hf * HCW + o
                                nc.tensor.matmul(
                                    ps[ib][gu][:], wslice(gu, ib, hf, o),
                                    xt_sb[:, hb],
                                    start=(hb == 0), stop=(hb == HB - 1),
                                )
                for ib in group:
                    sil = sil_pool.tile([P, T], f32, tag="sil")
                    nc.scalar.activation(sil[:], ps[ib][0][:], AF.Silu)
                    nc.vector.tensor_tensor(h_all[:, ib, :], sil[:],
                                            ps[ib][1][:], ALU.mult)

            # ---- phase 2: down projection (partial sums to DRAM) -------
            # (q, t) units of 2 PSUM banks, 4 units pipelined via the 8-slot
            # psum ring.  Out-writes own the sync ring; q2/q3 wd fetches ride
            # the scalar ring one q ahead (their slots free exactly then).
            for q in range(NQ if do2 else 0):
                if q == 1:
                    emit_wd_dmas(C, 2)
                if q == 2:
                    emit_wd_dmas(C, 3)
                wdq = wd_tiles[q]
                for t in range(TT):
                    ps_o = [
                        ps_pool.tile([P, 512], f32, tag="ps",
                                     name=f"ps_o_{q}_{t}_{hc}")
                        for hc in range(2)
                    ]
                    for hc in range(2):
                        for g0, sz, dt_ in wdq:
                            for k in range(sz):
                                ib = g0 + k
                                nc.tensor.matmul(
                                    ps_o[hc][:],
                                    h_all[:, ib, ts(t, P)],
                                    dt_[:, k, ds(hc * 512, 512)],
                                    start=(ib == 0),
                                    stop=(ib == IB - 1),
                                )
                    if q == NQ - 1 and t == TT - 1:
                        # Last unit: hc0 (done 11 MMs early) evacuates in
                        # 256-quarters under the hc1 MMs; hc1 in one half so
                        # the exposed tail is one ACT copy + one small DMA.
                        for hc4 in range(2):
                            otq = oev_pool.tile([P, 256], odt, tag="oevq",
                                                bufs=4, name=f"otq_{hc4}")
                            nc.scalar.copy(
                                otq[:], ps_o[0][:, ds(hc4 * 256, 256)])
                            eng = S if hc4 % 2 == 0 else C
                            eng.dma_start(
                                out[ds(t * P, P),
                                    ds(q * HQ + hc4 * 256, 256)], otq[:])
                        oth = oev_pool.tile([P, 512], odt, tag="oevq",
                                            bufs=4, name="oth_last")
                        nc.scalar.copy(oth[:], ps_o[1][:])
                        nc.sync.dma_start(
                            out[ds(t * P, P), ds(q * HQ + 512, 512)], oth[:])
                    else:
                        ot = oev_pool.tile([P, 2 * 512], odt, tag="oev",
                                           name=f"ot_{q}_{t}")
                        for hc in range(2):
                            # ACT copy: DVE tensor_copy measured faster in
                            # the cost model but hit NRT_EXEC_UNIT_
                            # UNRECOVERABLE on hardware; ACT is the
                            # verified-stable path.
                            nc.scalar.copy(ot[:, ds(hc * 512, 512)],
                                           ps_o[hc][:])
                        nc.sync.dma_start(
                            out[ds(t * P, P), ds(q * HQ, HQ)], ot[:]
                        )

            loop_cm.__exit__(None, None, None)

    nc.compile()  # bacc lowering: register alloc + multi-wait splitting
    return nc


def _prep_inputs(x, gate_weight, up_weight, down_weight, gate_scale, up_scale,
                 down_scale):
    """Dequantize + pad + shard + transpose on the host into per-core bf16
    DMA layouts (see module docstring)."""
    import ml_dtypes

    bf = ml_dtypes.bfloat16

    def deq_pad(w, s):
        w = np.asarray(w, np.float32)
        s = np.asarray(s, np.float32)
        wd_ = (w.reshape(I_FULL // P, P, HB, P) * s[:, None, :, None]).reshape(
            I_FULL, H
        ).astype(bf)
        wp = np.zeros((I_PAD, H), bf)
        wp[:I_FULL] = wd_
        return wp

    gw = deq_pad(gate_weight, gate_scale)
    uw = deq_pad(up_weight, up_scale)
    dw = deq_pad(down_weight, down_scale)

    x = np.asarray(x, np.float32).astype(bf)
    # xt[p, hb, t] = x[t, hb*128+p]
    xt = np.ascontiguousarray(x.reshape(T, HB, P).transpose(2, 1, 0))

    in_maps = []
    for c in range(NCORES):
        i0 = c * I_CORE

        # [ib, i, hb', p] -> [ib, hb, p, i] -> [ib, hf, p, o, i]
        def gu_prep(wc):
            a = wc.reshape(IB, P, HB, P).transpose(0, 2, 3, 1)
            a = a.reshape(IB, 2, HCW, P, P).transpose(0, 1, 3, 2, 4)
            return a

        g5 = gu_prep(gw[i0: i0 + I_CORE])
        u5 = gu_prep(uw[i0: i0 + I_CORE])
        wgu_prep = np.ascontiguousarray(
            np.stack([g5, u5], axis=1)  # [ib, gu, hf, p, o, i]
        )
        # down: [q, p, ib, j] = w[ib*128+p, q*1024+j]
        wd_prep = np.ascontiguousarray(
            dw[i0: i0 + I_CORE].reshape(IB, P, NQ, HQ).transpose(2, 1, 0, 3)
        )
        in_maps.append({"xt": xt, "wgu": wgu_prep, "wd": wd_prep})
    return in_maps


def kernel(x, gate_weight, up_weight, down_weight, gate_scale, up_scale,
           down_scale, blocksize):
    global LAST_RESULTS
    assert int(blocksize) == P, f"kernel hardcodes blocksize=128, got {blocksize}"

    from concourse.bass_utils import run_bass_kernel_spmd

    trace = os.environ.get("BASS_TRACE", "0") == "1"

    nc = _PROG_CACHE.get(1)
    if nc is None:
        nc = _build_program()
        _PROG_CACHE[1] = nc
    in_maps = _prep_inputs(
        x, gate_weight, up_weight, down_weight, gate_scale, up_scale, down_scale
    )
    results = run_bass_kernel_spmd(
        nc, in_maps, core_ids=list(range(NCORES)), trace=trace
    )
    LAST_RESULTS = results

    acc = np.zeros((T, H), np.float64)
    for res in results.results:
        acc += np.asarray(res["out"], np.float64)
    return acc.astype(np.float32)
